# revision 4
# baseline (speedup 1.0000x reference)
"""Trainium2 Bass kernel for nn_AE2TK_15925738734149 (moe_routing).

Strategy: data-parallel over the batch dim (B=8 -> 8 NeuronCores, one
batch row per core). All activations are kept feature-major ([H, T]) on
device so every weight matrix is consumed as the matmul's stationary
lhsT operand with zero on-device transposes. Dropout masks come from a
fixed jax PRNG key (42) independent of the inputs, so they are
precomputed host-side once and shipped as scaled bf16 tensors. The
routing blend, up-projection biases and the recon scalar are exact 0/1
selections / cheap reductions, done host-side in f32 from the two
expert outputs.
"""

import functools
import os

import numpy as np
import ml_dtypes

# ---------------------------------------------------------------------------
# Environment patches (needed before building/running the Bass kernel)
# ---------------------------------------------------------------------------


def _install_ntff_hook():
    """Best-effort: register the axon NTFF profile hook so trace=True /
    BASS_TRACE=1 can report HW exec time. Harmless no-op on failure."""
    try:
        import sys
        import types

        try:
            import antenv.axon_hooks  # noqa: F401
        except ImportError:
            import antenv

            mod = types.ModuleType("antenv.axon_hooks")
            mod._hook = None

            def set_axon_ntff_profile_hook(hook):
                mod._hook = hook

            def get_axon_ntff_profile_hook():
                return mod._hook

            mod.set_axon_ntff_profile_hook = set_axon_ntff_profile_hook
            mod.get_axon_ntff_profile_hook = get_axon_ntff_profile_hook
            sys.modules["antenv.axon_hooks"] = mod
            antenv.axon_hooks = mod

        from antenv.axon_hooks import (
            get_axon_ntff_profile_hook,
            set_axon_ntff_profile_hook,
        )

        if get_axon_ntff_profile_hook() is None:
            from trn_agent_boot.trn_boot import _ntff_profile_via_ctypes

            so_path = "/opt/axon/libaxon_pjrt.so"
            if os.path.exists(so_path):
                hook = _ntff_profile_via_ctypes(so_path)
                if hook is not None:
                    set_axon_ntff_profile_hook(hook)
    except Exception:
        pass


_install_ntff_hook()

# ---------------------------------------------------------------------------
# Problem constants (hardcoded; kernel.py must be self-contained)
# ---------------------------------------------------------------------------

B, S, H = 8, 2048, 1024
DIMS = (512, 256)  # expert hidden dims
KEEP = 0.8
NCORES = 8
P = 128  # SBUF partitions
CH = 512  # matmul moving-operand chunk (one PSUM bank of f32)
NCH = S // CH

_bf16 = ml_dtypes.bfloat16


# ---------------------------------------------------------------------------
# Host-side dropout masks (fixed key 42, input-independent)
# ---------------------------------------------------------------------------


@functools.lru_cache(maxsize=1)
def _dropout_masks():
    """Reproduce reference's dropout bernoulli draws exactly.

    IMPORTANT: the default PRNG impl in this environment is `rbg`, whose
    bit stream is backend-dependent. The graded reference executes on the
    default jax backend, so the draws here must too (no device pinning).

    Returns {(expert, which): mask array}, which in {"h","s0","s1"};
    mask is float32 in {0, 1/KEEP} with shape [B, S, d]."""
    import jax

    dkey = jax.random.key(42)
    k0, k1 = jax.random.split(dkey, 2)
    raw = {}
    for e, (kk, d) in enumerate(zip((k0, k1), DIMS)):
        ka, kb, kc = jax.random.split(kk, 3)
        raw[(e, "h")] = jax.random.bernoulli(ka, KEEP, (B, S, d))
        raw[(e, "s0")] = jax.random.bernoulli(kb, KEEP, (B, S, d // 2))
        raw[(e, "s1")] = jax.random.bernoulli(kc, KEEP, (B, S, d // 2))
    scale = np.float32(1.0 / KEEP)
    return {k: np.asarray(v).astype(np.float32) * scale for k, v in raw.items()}


@functools.lru_cache(maxsize=4)
def _mask_shards(e, which):
    """Per-core transposed scaled bf16 mask shards: list of [d, S]."""
    m = _dropout_masks()[(e, which)]
    return [np.ascontiguousarray(m[b].T).astype(_bf16) for b in range(B)]


# ---------------------------------------------------------------------------
# Bass kernel builder
# ---------------------------------------------------------------------------

_NC_CACHE = {}


def _build_nc(choices):
    """Build the per-core Bass graph. choices = (c0, c1), c in {0,1,2}:
    0/1 pick that sub-AE, 2 is the identity branch (skip sub matmuls)."""
    import concourse.bacc as bacc
    import concourse.mybir as mybir
    import concourse.tile as tile

    f32 = mybir.dt.float32
    bf16 = mybir.dt.bfloat16
    Tanh = mybir.ActivationFunctionType.Tanh

    nc = bacc.Bacc()

    hsT_ext = nc.declare_dram_parameter("hsT", [H, S], bf16, isOutput=False)

    prm = {}
    for e, d in enumerate(DIMS):
        prm[(e, "dw")] = nc.declare_dram_parameter(f"e{e}_dw", [H, d], bf16, False)
        prm[(e, "db")] = nc.declare_dram_parameter(f"e{e}_db", [d, 1], f32, False)
        prm[(e, "m1")] = nc.declare_dram_parameter(f"e{e}_m1", [d, S], bf16, False)
        if choices[e] in (0, 1):
            prm[(e, "sdw")] = nc.declare_dram_parameter(
                f"e{e}_sdw", [d, d // 2], bf16, False
            )
            prm[(e, "sdb")] = nc.declare_dram_parameter(
                f"e{e}_sdb", [d // 2, 1], f32, False
            )
            prm[(e, "sm")] = nc.declare_dram_parameter(
                f"e{e}_sm", [d // 2, S], bf16, False
            )
            prm[(e, "suw")] = nc.declare_dram_parameter(
                f"e{e}_suw", [d // 2, d], bf16, False
            )
            prm[(e, "sub")] = nc.declare_dram_parameter(f"e{e}_sub", [d, 1], f32, False)
        prm[(e, "uw")] = nc.declare_dram_parameter(f"e{e}_uw", [d, H], bf16, False)
        prm[(e, "out")] = nc.declare_dram_parameter(f"h{e}T", [H, S], bf16, True)

    with tile.TileContext(nc) as tc:
        with (
            tc.tile_pool(name="hs", bufs=1) as hs_pool,
            tc.tile_pool(name="wts", bufs=1) as wts,
            tc.tile_pool(name="acts", bufs=1) as acts,
            tc.tile_pool(name="outs", bufs=4) as outs,
            tc.tile_pool(name="psum", bufs=2, space="PSUM") as psum,
        ):
            hs_sb = hs_pool.tile([P, H // P, S], bf16, tag="hs")
            nc.sync.dma_start(hs_sb[:], hsT_ext.rearrange("(t p) n -> p t n", p=P))

            def proj_tanh_mask(src_sb, w_sb, b_sb, m_sb, dst_sb, kt, mt):
                """dst = tanh(src @ w + b) * m, feature-major tiles.
                src_sb [P, kt, S], w_sb [P, kt, mt*P], dst_sb [P, mt, S]."""
                for m in range(mt):
                    ps = psum.tile([P, NCH, CH], f32, tag="ps")
                    for c in range(NCH):
                        for k in range(kt):
                            nc.tensor.matmul(
                                ps[:, c, :],
                                w_sb[:, k, m * P : (m + 1) * P],
                                src_sb[:, k, c * CH : (c + 1) * CH],
                                start=(k == 0),
                                stop=(k == kt - 1),
                            )
                    nc.scalar.activation(
                        dst_sb[:, m, :],
                        ps.rearrange("p c n -> p (c n)"),
                        Tanh,
                        bias=b_sb[:, m, :],
                    )
                    nc.vector.tensor_mul(dst_sb[:, m, :], dst_sb[:, m, :], m_sb[:, m, :])

            for e, d in enumerate(DIMS):
                kd = d // P
                kd2 = (d // 2) // P
                choice = choices[e]

                dw_sb = wts.tile([P, H // P, d], bf16, tag="dw")
                nc.sync.dma_start(
                    dw_sb[:], prm[(e, "dw")].rearrange("(t p) m -> p t m", p=P)
                )
                db_sb = wts.tile([P, kd, 1], f32, tag="db")
                nc.sync.dma_start(
                    db_sb[:], prm[(e, "db")].rearrange("(t p) o -> p t o", p=P)
                )
                m1_sb = acts.tile([P, kd, S], bf16, tag="m1")
                nc.sync.dma_start(
                    m1_sb[:], prm[(e, "m1")].rearrange("(t p) n -> p t n", p=P)
                )
                h_sb = acts.tile([P, kd, S], bf16, tag="h")

                proj_tanh_mask(hs_sb, dw_sb, db_sb, m1_sb, h_sb, H // P, kd)

                if choice in (0, 1):
                    sdw_sb = wts.tile([P, kd, d // 2], bf16, tag="sdw")
                    nc.sync.dma_start(
                        sdw_sb[:], prm[(e, "sdw")].rearrange("(t p) m -> p t m", p=P)
                    )
                    sdb_sb = wts.tile([P, kd2, 1], f32, tag="sdb")
                    nc.sync.dma_start(
                        sdb_sb[:], prm[(e, "sdb")].rearrange("(t p) o -> p t o", p=P)
                    )
                    sm_sb = acts.tile([P, kd2, S], bf16, tag="sm")
                    nc.sync.dma_start(
                        sm_sb[:], prm[(e, "sm")].rearrange("(t p) n -> p t n", p=P)
                    )
                    t_sb = acts.tile([P, kd2, S], bf16, tag="t")

                    proj_tanh_mask(h_sb, sdw_sb, sdb_sb, sm_sb, t_sb, kd, kd2)

                    suw_sb = wts.tile([P, kd2, d], bf16, tag="suw")
                    nc.sync.dma_start(
                        suw_sb[:], prm[(e, "suw")].rearrange("(t p) m -> p t m", p=P)
                    )
                    sub_sb = wts.tile([P, kd, 1], f32, tag="sub")
                    nc.sync.dma_start(
                        sub_sb[:], prm[(e, "sub")].rearrange("(t p) o -> p t o", p=P)
                    )
                    s_sb = acts.tile([P, kd, S], bf16, tag="s")

                    # s = tanh(t @ suw + sub)  (no dropout on the sub output)
                    for m in range(kd):
                        ps = psum.tile([P, NCH, CH], f32, tag="ps")
                        for c in range(NCH):
                            for k in range(kd2):
                                nc.tensor.matmul(
                                    ps[:, c, :],
                                    suw_sb[:, k, m * P : (m + 1) * P],
                                    t_sb[:, k, c * CH : (c + 1) * CH],
                                    start=(k == 0),
                                    stop=(k == kd2 - 1),
                                )
                        nc.scalar.activation(
                            s_sb[:, m, :],
                            ps.rearrange("p c n -> p (c n)"),
                            Tanh,
                            bias=sub_sb[:, m, :],
                        )
                    blend_sb = s_sb
                else:
                    blend_sb = h_sb

                uw_sb = wts.tile([P, kd, H], bf16, tag="uw")
                nc.sync.dma_start(
                    uw_sb[:], prm[(e, "uw")].rearrange("(t p) m -> p t m", p=P)
                )
                out_ext = prm[(e, "out")].rearrange("(t p) n -> t p n", p=P)

                # out = blend @ uw  (up bias added host-side)
                for m in range(H // P):
                    ps = psum.tile([P, NCH, CH], f32, tag="ps")
                    for c in range(NCH):
                        for k in range(kd):
                            nc.tensor.matmul(
                                ps[:, c, :],
                                uw_sb[:, k, m * P : (m + 1) * P],
                                blend_sb[:, k, c * CH : (c + 1) * CH],
                                start=(k == 0),
                                stop=(k == kd - 1),
                            )
                    o_sb = outs.tile([P, S], bf16, tag="o")
                    nc.any.tensor_copy(o_sb[:], ps.rearrange("p c n -> p (c n)"))
                    nc.sync.dma_start(out_ext[m], o_sb[:])

    nc.finalize()
    return nc


# ---------------------------------------------------------------------------
# kernel() entry point
# ---------------------------------------------------------------------------


def kernel(**inputs):
    x = np.asarray(inputs["x"], dtype=np.float32)
    noise = np.asarray(inputs["noise"], dtype=np.float32)
    route = np.asarray(inputs["route"]).astype(np.int64).reshape(S)
    sub_choice = np.asarray(inputs["sub_choice"]).astype(np.int64).reshape(2)
    choices = (int(sub_choice[0]), int(sub_choice[1]))

    w = {
        k: np.asarray(v, dtype=np.float32)
        for k, v in inputs.items()
        if k not in ("x", "noise", "route", "sub_choice")
    }

    hs = x + np.float32(0.002) * noise  # [B, S, H] f32 exact

    key = choices
    if key not in _NC_CACHE:
        _NC_CACHE[key] = _build_nc(choices)
    nc = _NC_CACHE[key]

    in_maps = []
    for b in range(NCORES):
        m = {"hsT": np.ascontiguousarray(hs[b].T).astype(_bf16)}
        for e in range(2):
            m[f"e{e}_dw"] = w[f"e{e}_dw"].astype(_bf16)
            m[f"e{e}_db"] = w[f"e{e}_db"].reshape(-1, 1)
            m[f"e{e}_m1"] = _mask_shards(e, "h")[b]
            c = choices[e]
            if c in (0, 1):
                m[f"e{e}_sdw"] = w[f"e{e}_s{c}_dw"].astype(_bf16)
                m[f"e{e}_sdb"] = w[f"e{e}_s{c}_db"].reshape(-1, 1)
                m[f"e{e}_sm"] = _mask_shards(e, f"s{c}")[b]
                m[f"e{e}_suw"] = w[f"e{e}_s{c}_uw"].astype(_bf16)
                m[f"e{e}_sub"] = w[f"e{e}_s{c}_ub"].reshape(-1, 1)
            m[f"e{e}_uw"] = w[f"e{e}_uw"].astype(_bf16)
        in_maps.append(m)

    from concourse.bass_utils import run_bass_kernel_spmd

    trace = bool(os.environ.get("KERNEL_PROFILE"))
    res = run_bass_kernel_spmd(
        nc, in_maps, core_ids=list(range(NCORES)), trace=trace
    )
    kernel._last_exec_time_ns = res.exec_time_ns
    kernel._last_results = res

    # Gather + host epilogue: up biases, routing blend, recon.
    h0 = np.empty((B, S, H), dtype=np.float32)
    h1 = np.empty((B, S, H), dtype=np.float32)
    for b in range(NCORES):
        h0[b] = res.results[b]["h0T"].astype(np.float32).T
        h1[b] = res.results[b]["h1T"].astype(np.float32).T
    h0 += w["e0_ub"][None, None, :]
    h1 += w["e1_ub"][None, None, :]

    opt = np.zeros((B, S, H), dtype=np.float32)
    sel0 = route == 0
    sel1 = route == 1
    sel2 = route == 2
    opt[:, sel0] = h0[:, sel0]
    opt[:, sel1] = h1[:, sel1]
    opt[:, sel2] = hs[:, sel2]

    d0 = (hs - h0).astype(np.float64)
    d1 = (hs - h1).astype(np.float64)
    recon = np.float32((np.mean(d0 * d0) + np.mean(d1 * d1)) / 2.0)

    return opt, recon


kernel._last_exec_time_ns = None
kernel._last_results = None


# revision 6
# speedup vs baseline: 1.0482x; 1.0482x over previous
"""Trainium2 Bass kernel for nn_AE2TK_15925738734149 (moe_routing).

Strategy: data-parallel over the batch dim (B=8 -> 8 NeuronCores, one
batch row per core). All activations are kept feature-major ([H, T]) on
device so every weight matrix is consumed as the matmul's stationary
lhsT operand with zero on-device transposes. Dropout masks come from a
fixed jax PRNG key (42) independent of the inputs, so they are
precomputed host-side once and shipped as scaled bf16 tensors. The
routing blend, up-projection biases and the recon scalar are exact 0/1
selections / cheap reductions, done host-side in f32 from the two
expert outputs.
"""

import functools
import os

import numpy as np
import ml_dtypes

# ---------------------------------------------------------------------------
# Environment patches (needed before building/running the Bass kernel)
# ---------------------------------------------------------------------------


def _install_ntff_hook():
    """Best-effort: register the axon NTFF profile hook so trace=True /
    BASS_TRACE=1 can report HW exec time. Harmless no-op on failure."""
    try:
        import sys
        import types

        try:
            import antenv.axon_hooks  # noqa: F401
        except ImportError:
            import antenv

            mod = types.ModuleType("antenv.axon_hooks")
            mod._hook = None

            def set_axon_ntff_profile_hook(hook):
                mod._hook = hook

            def get_axon_ntff_profile_hook():
                return mod._hook

            mod.set_axon_ntff_profile_hook = set_axon_ntff_profile_hook
            mod.get_axon_ntff_profile_hook = get_axon_ntff_profile_hook
            sys.modules["antenv.axon_hooks"] = mod
            antenv.axon_hooks = mod

        from antenv.axon_hooks import (
            get_axon_ntff_profile_hook,
            set_axon_ntff_profile_hook,
        )

        if get_axon_ntff_profile_hook() is None:
            from trn_agent_boot.trn_boot import _ntff_profile_via_ctypes

            so_path = "/opt/axon/libaxon_pjrt.so"
            if os.path.exists(so_path):
                hook = _ntff_profile_via_ctypes(so_path)
                if hook is not None:
                    set_axon_ntff_profile_hook(hook)
    except Exception:
        pass


_install_ntff_hook()

# ---------------------------------------------------------------------------
# Problem constants (hardcoded; kernel.py must be self-contained)
# ---------------------------------------------------------------------------

B, S, H = 8, 2048, 1024
DIMS = (512, 256)  # expert hidden dims
KEEP = 0.8
NCORES = 8
P = 128  # SBUF partitions
CH = 512  # matmul moving-operand chunk (one PSUM bank of f32)
NCH = S // CH

_bf16 = ml_dtypes.bfloat16


# ---------------------------------------------------------------------------
# Host-side dropout masks (fixed key 42, input-independent)
# ---------------------------------------------------------------------------


@functools.lru_cache(maxsize=1)
def _dropout_masks():
    """Reproduce reference's dropout bernoulli draws exactly.

    IMPORTANT: the default PRNG impl in this environment is `rbg`, whose
    bit stream is backend-dependent. The graded reference executes on the
    default jax backend, so the draws here must too (no device pinning).

    Returns {(expert, which): mask array}, which in {"h","s0","s1"};
    mask is float32 in {0, 1/KEEP} with shape [B, S, d]."""
    import jax

    dkey = jax.random.key(42)
    k0, k1 = jax.random.split(dkey, 2)
    raw = {}
    for e, (kk, d) in enumerate(zip((k0, k1), DIMS)):
        ka, kb, kc = jax.random.split(kk, 3)
        raw[(e, "h")] = jax.random.bernoulli(ka, KEEP, (B, S, d))
        raw[(e, "s0")] = jax.random.bernoulli(kb, KEEP, (B, S, d // 2))
        raw[(e, "s1")] = jax.random.bernoulli(kc, KEEP, (B, S, d // 2))
    scale = np.float32(1.0 / KEEP)
    return {k: np.asarray(v).astype(np.float32) * scale for k, v in raw.items()}


@functools.lru_cache(maxsize=4)
def _mask_shards(e, which):
    """Per-core transposed scaled bf16 mask shards: list of [d, S]."""
    m = _dropout_masks()[(e, which)]
    return [np.ascontiguousarray(m[b].T).astype(_bf16) for b in range(B)]


# ---------------------------------------------------------------------------
# Bass kernel builder
# ---------------------------------------------------------------------------

_NC_CACHE = {}


def _build_nc(choices):
    """Build the per-core Bass graph. choices = (c0, c1), c in {0,1,2}:
    0/1 pick that sub-AE, 2 is the identity branch (skip sub matmuls)."""
    import concourse.bacc as bacc
    import concourse.mybir as mybir
    import concourse.tile as tile

    f32 = mybir.dt.float32
    bf16 = mybir.dt.bfloat16
    Tanh = mybir.ActivationFunctionType.Tanh

    nc = bacc.Bacc()

    hsT_ext = nc.declare_dram_parameter("hsT", [H, S], bf16, isOutput=False)

    prm = {}
    for e, d in enumerate(DIMS):
        prm[(e, "dw")] = nc.declare_dram_parameter(f"e{e}_dw", [H, d], bf16, False)
        prm[(e, "db")] = nc.declare_dram_parameter(f"e{e}_db", [d, 1], f32, False)
        prm[(e, "m1")] = nc.declare_dram_parameter(f"e{e}_m1", [d, S], bf16, False)
        if choices[e] in (0, 1):
            prm[(e, "sdw")] = nc.declare_dram_parameter(
                f"e{e}_sdw", [d, d // 2], bf16, False
            )
            prm[(e, "sdb")] = nc.declare_dram_parameter(
                f"e{e}_sdb", [d // 2, 1], f32, False
            )
            prm[(e, "sm")] = nc.declare_dram_parameter(
                f"e{e}_sm", [d // 2, S], bf16, False
            )
            prm[(e, "suw")] = nc.declare_dram_parameter(
                f"e{e}_suw", [d // 2, d], bf16, False
            )
            prm[(e, "sub")] = nc.declare_dram_parameter(f"e{e}_sub", [d, 1], f32, False)
        prm[(e, "uw")] = nc.declare_dram_parameter(f"e{e}_uw", [d, H], bf16, False)
        prm[(e, "out")] = nc.declare_dram_parameter(f"h{e}T", [H, S], bf16, True)

    with tile.TileContext(nc) as tc:
        with (
            tc.tile_pool(name="hs", bufs=1) as hs_pool,
            tc.tile_pool(name="wts", bufs=1) as wts,
            tc.tile_pool(name="acts", bufs=1) as acts,
            tc.tile_pool(name="outs", bufs=4) as outs,
            tc.tile_pool(name="psum", bufs=2, space="PSUM") as psum,
        ):
            hs_sb = hs_pool.tile([P, H // P, S], bf16, tag="hs")
            hsT_r = hsT_ext.rearrange("(t p) n -> p t n", p=P)
            for k in range(H // P):
                nc.sync.dma_start(hs_sb[:, k, :], hsT_r[:, k, :])

            def load_ktiled(pool, ext, kt, cols, dtype, tag):
                """DMA [kt*P, cols] dram tensor into [P, kt, cols] SBUF,
                one DMA per k-tile so consumers start early."""
                sb = pool.tile([P, kt, cols], dtype, tag=tag)
                r = ext.rearrange("(t p) n -> p t n", p=P)
                for k in range(kt):
                    nc.sync.dma_start(sb[:, k, :], r[:, k, :])
                return sb

            def proj_tanh_mask(src_sb, w_sb, b_sb, m_sb, dst_sb, kt, mt):
                """dst = tanh(src @ w + b) * m, feature-major tiles.
                src_sb [P, kt, S], w_sb [P, kt, mt*P], dst_sb [P, mt, S].
                k outer, chunks inner: one weight load serves NCH matmuls."""
                for m in range(mt):
                    ps = psum.tile([P, NCH, CH], f32, tag="ps")
                    for k in range(kt):
                        for c in range(NCH):
                            nc.tensor.matmul(
                                ps[:, c, :],
                                w_sb[:, k, m * P : (m + 1) * P],
                                src_sb[:, k, c * CH : (c + 1) * CH],
                                start=(k == 0),
                                stop=(k == kt - 1),
                            )
                    nc.scalar.activation(
                        dst_sb[:, m, :],
                        ps.rearrange("p c n -> p (c n)"),
                        Tanh,
                        bias=b_sb[:, m, :],
                    )
                    nc.vector.tensor_mul(dst_sb[:, m, :], dst_sb[:, m, :], m_sb[:, m, :])

            for e, d in enumerate(DIMS):
                kd = d // P
                kd2 = (d // 2) // P
                choice = choices[e]

                dw_sb = load_ktiled(wts, prm[(e, "dw")], H // P, d, bf16, "dw")
                db_sb = wts.tile([P, kd, 1], f32, tag="db")
                nc.sync.dma_start(
                    db_sb[:], prm[(e, "db")].rearrange("(t p) o -> p t o", p=P)
                )
                m1_sb = load_ktiled(acts, prm[(e, "m1")], kd, S, bf16, "m1")
                h_sb = acts.tile([P, kd, S], bf16, tag="h")

                proj_tanh_mask(hs_sb, dw_sb, db_sb, m1_sb, h_sb, H // P, kd)

                if choice in (0, 1):
                    sdw_sb = load_ktiled(wts, prm[(e, "sdw")], kd, d // 2, bf16, "sdw")
                    sdb_sb = wts.tile([P, kd2, 1], f32, tag="sdb")
                    nc.sync.dma_start(
                        sdb_sb[:], prm[(e, "sdb")].rearrange("(t p) o -> p t o", p=P)
                    )
                    sm_sb = load_ktiled(acts, prm[(e, "sm")], kd2, S, bf16, "sm")
                    t_sb = acts.tile([P, kd2, S], bf16, tag="t")

                    proj_tanh_mask(h_sb, sdw_sb, sdb_sb, sm_sb, t_sb, kd, kd2)

                    suw_sb = load_ktiled(wts, prm[(e, "suw")], kd2, d, bf16, "suw")
                    sub_sb = wts.tile([P, kd, 1], f32, tag="sub")
                    nc.sync.dma_start(
                        sub_sb[:], prm[(e, "sub")].rearrange("(t p) o -> p t o", p=P)
                    )
                    s_sb = acts.tile([P, kd, S], bf16, tag="s")

                    # s = tanh(t @ suw + sub)  (no dropout on the sub output)
                    for m in range(kd):
                        ps = psum.tile([P, NCH, CH], f32, tag="ps")
                        for k in range(kd2):
                            for c in range(NCH):
                                nc.tensor.matmul(
                                    ps[:, c, :],
                                    suw_sb[:, k, m * P : (m + 1) * P],
                                    t_sb[:, k, c * CH : (c + 1) * CH],
                                    start=(k == 0),
                                    stop=(k == kd2 - 1),
                                )
                        nc.scalar.activation(
                            s_sb[:, m, :],
                            ps.rearrange("p c n -> p (c n)"),
                            Tanh,
                            bias=sub_sb[:, m, :],
                        )
                    blend_sb = s_sb
                else:
                    blend_sb = h_sb

                uw_sb = load_ktiled(wts, prm[(e, "uw")], kd, H, bf16, "uw")
                out_ext = prm[(e, "out")].rearrange("(t p) n -> t p n", p=P)

                # out = blend @ uw  (up bias added host-side); evacuate and
                # DMA per half so the kernel tail pipelines.
                for m in range(H // P):
                    ps = psum.tile([P, NCH, CH], f32, tag="ps")
                    for k in range(kd):
                        for c in range(NCH):
                            nc.tensor.matmul(
                                ps[:, c, :],
                                uw_sb[:, k, m * P : (m + 1) * P],
                                blend_sb[:, k, c * CH : (c + 1) * CH],
                                start=(k == 0),
                                stop=(k == kd - 1),
                            )
                    o_sb = outs.tile([P, S], bf16, tag="o")
                    psf = ps.rearrange("p c n -> p (c n)")
                    half = S // 2
                    for hf in range(2):
                        nc.any.tensor_copy(
                            o_sb[:, hf * half : (hf + 1) * half],
                            psf[:, hf * half : (hf + 1) * half],
                        )
                        nc.sync.dma_start(
                            out_ext[m, :, hf * half : (hf + 1) * half],
                            o_sb[:, hf * half : (hf + 1) * half],
                        )

    nc.finalize()
    return nc


# ---------------------------------------------------------------------------
# kernel() entry point
# ---------------------------------------------------------------------------


def kernel(**inputs):
    x = np.asarray(inputs["x"], dtype=np.float32)
    noise = np.asarray(inputs["noise"], dtype=np.float32)
    route = np.asarray(inputs["route"]).astype(np.int64).reshape(S)
    sub_choice = np.asarray(inputs["sub_choice"]).astype(np.int64).reshape(2)
    choices = (int(sub_choice[0]), int(sub_choice[1]))

    w = {
        k: np.asarray(v, dtype=np.float32)
        for k, v in inputs.items()
        if k not in ("x", "noise", "route", "sub_choice")
    }

    hs = x + np.float32(0.002) * noise  # [B, S, H] f32 exact

    key = choices
    if key not in _NC_CACHE:
        _NC_CACHE[key] = _build_nc(choices)
    nc = _NC_CACHE[key]

    in_maps = []
    for b in range(NCORES):
        m = {"hsT": np.ascontiguousarray(hs[b].T).astype(_bf16)}
        for e in range(2):
            m[f"e{e}_dw"] = w[f"e{e}_dw"].astype(_bf16)
            m[f"e{e}_db"] = w[f"e{e}_db"].reshape(-1, 1)
            m[f"e{e}_m1"] = _mask_shards(e, "h")[b]
            c = choices[e]
            if c in (0, 1):
                m[f"e{e}_sdw"] = w[f"e{e}_s{c}_dw"].astype(_bf16)
                m[f"e{e}_sdb"] = w[f"e{e}_s{c}_db"].reshape(-1, 1)
                m[f"e{e}_sm"] = _mask_shards(e, f"s{c}")[b]
                m[f"e{e}_suw"] = w[f"e{e}_s{c}_uw"].astype(_bf16)
                m[f"e{e}_sub"] = w[f"e{e}_s{c}_ub"].reshape(-1, 1)
            m[f"e{e}_uw"] = w[f"e{e}_uw"].astype(_bf16)
        in_maps.append(m)

    from concourse.bass_utils import run_bass_kernel_spmd

    trace = bool(os.environ.get("KERNEL_PROFILE"))
    res = run_bass_kernel_spmd(
        nc, in_maps, core_ids=list(range(NCORES)), trace=trace
    )
    kernel._last_exec_time_ns = res.exec_time_ns
    kernel._last_results = res

    # Gather + host epilogue: up biases, routing blend, recon.
    h0 = np.empty((B, S, H), dtype=np.float32)
    h1 = np.empty((B, S, H), dtype=np.float32)
    for b in range(NCORES):
        h0[b] = res.results[b]["h0T"].astype(np.float32).T
        h1[b] = res.results[b]["h1T"].astype(np.float32).T
    h0 += w["e0_ub"][None, None, :]
    h1 += w["e1_ub"][None, None, :]

    opt = np.zeros((B, S, H), dtype=np.float32)
    sel0 = route == 0
    sel1 = route == 1
    sel2 = route == 2
    opt[:, sel0] = h0[:, sel0]
    opt[:, sel1] = h1[:, sel1]
    opt[:, sel2] = hs[:, sel2]

    d0 = (hs - h0).astype(np.float64)
    d1 = (hs - h1).astype(np.float64)
    recon = np.float32((np.mean(d0 * d0) + np.mean(d1 * d1)) / 2.0)

    return opt, recon


kernel._last_exec_time_ns = None
kernel._last_results = None
